# revision 43
# baseline (speedup 1.0000x reference)
"""Trainium2 Bass kernel for nn_MultiHeadAttention_34162169872901.

MultiHeadAttention (B=4, S=2048, d_model=512, 8 heads, d_k=64) with a
relative-position bias table (511 entries, clamp +-255) and an all-ones mask.

Sharding (8 NeuronCores): core c handles batch b = c//2 and 4 of the 8 heads
(c%2 selects the head half) -- data parallel on B, tensor parallel on heads.
Each core computes its 4 heads' Q/K/V projections, the full attention for its
batch, normalization, and its partial output projection; the host sums the two
partial outputs per batch (and adds the output bias bo).

v4 -- scheduled for the ACT (exp) bottleneck:
  - The softmax exp on the scalar/ACT engine is the binding resource
    (128 x [128,1024] exps ~= 1.13us each).  The schedule starts exp as
    early as possible and tries to never let ACT starve.
  - DMA triggers cost ~650ns each and serialize per engine, so inputs are
    consolidated into a few multi-block tiles, triggered first thing in
    the program, in need order, on the two fast queues (sync + gpsimd).
  - Score matmuls are emitted ah-major so the quadrant whose PSUM bank
    frees first is refilled without waiting for the second exp.
  - AV matmuls are globally deferred by two groups (and carry across
    block boundaries) so filler never blocks the score stream and the
    next block's first scores precede the previous block's AV drain.
  - The relative-bias exp-table is a [128, 4, 1408] sliding-window table
    (every in-band Toeplitz block is a contiguous 512-col slice).
  - PSUM: scores ring 2x[128,1024] (4 banks), AV-accumulator ring
    2x[65,512] (2 banks), epilogue/projection ring 2x[128,512] (2 banks).
  - Per-(u,hp) normalization and the O-projection are pipelined into the
    following attention block; the final O-projection is split so half is
    staged during the last block, with keep-warm matmuls bridging the
    final normalization chain.
"""

import sys
import types

import numpy as np

B = 4
S = 2048
D = 512
NHEAD = 8
DK = 64
NCORES = 8
MAX_REL = 255
NKT = S // 128   # 16 k-tiles
NU = S // 512    # 4 q-units
NG = NKT // 2    # 8 score groups per (u, hp)


def _install_axon_hooks():
    """Provide antenv.axon_hooks (missing in this image) so bass_utils'
    trace path can be used; harmless when tracing is off."""
    try:
        import antenv
    except ImportError:
        return
    try:
        from antenv.axon_hooks import get_axon_ntff_profile_hook  # noqa: F401
        return
    except ImportError:
        pass
    hook = None
    try:
        from trn_agent_boot.trn_boot import _ntff_profile_via_ctypes
        hook = _ntff_profile_via_ctypes("/opt/axon/libaxon_pjrt.so")
    except Exception:
        hook = None
    m = types.ModuleType("antenv.axon_hooks")
    m.get_axon_ntff_profile_hook = lambda: hook
    m.set_axon_ntff_profile_hook = lambda h: None
    sys.modules["antenv.axon_hooks"] = m
    antenv.axon_hooks = m


_install_axon_hooks()

import concourse.bass as bass  # noqa: E402
import concourse.bacc as bacc  # noqa: E402
import concourse.mybir as mybir  # noqa: E402
from concourse import tile  # noqa: E402
from concourse.bass_utils import run_bass_kernel_spmd  # noqa: E402
from concourse.vector_clock import ScopedClock as _ScopedClock  # noqa: E402

f32 = mybir.dt.float32
bf16 = mybir.dt.bfloat16
f16 = mybir.dt.float16
AF = mybir.ActivationFunctionType


def _patched_drain_and_barrier(self, tick_clock, wait_clock):
    # walrus in this container rejects >2 sem waits on one instruction; emit
    # the tail-drain waits as standalone wait instructions instead.
    nc = self.nc
    dummy = mybir.InstNoOp(name="drain-wait-probe", engine=mybir.EngineType.SP)
    wait_clock.add_sem_waits(dummy, _ScopedClock({None: tick_clock.global_clock}))
    handles = {h.name: h for h in self.sems.allocated().values()}
    si = dummy.sync_info
    for w in (si.on_wait if si is not None else []):
        nc.sync.wait_ge(handles[w.ant_name], w.wait_value)
    nc.sync.drain()
    nc.all_engine_barrier()
    popped = nc._tile_sem_poison_stack.pop()
    assert popped is self._sem_poison
    nc.clear_and_free_semaphores(list(self.sems.allocated().values()))
    nc.all_engine_barrier()


tile.TileContext._drain_and_barrier = _patched_drain_and_barrier


def _delta(t, u):
    # key-tile offset minus query-chunk offset; bias entry index is
    # delta + (p - f) + 255 clipped to [0, 510]
    return 128 * t - 512 * u


def _cls(t, u):
    d = _delta(t, u)
    if d <= -384:
        return 1  # whole block clamps to table[0]
    if d >= 768:
        return 2  # whole block clamps to table[510]
    return 0      # in-band: needs the Toeplitz block


def _ebt_col(t, u):
    # start column of the [128,512] in-band block inside the 1408-wide
    # sliding-window exp-bias table: 640 - delta
    return 640 - _delta(t, u)


def _gorder(u):
    # out-of-band groups first (no bias-table dependency, constant folded
    # into the exp bias).  For u == 0, order by K-projection availability:
    # high k-columns first, in-band groups in descending g.
    if u == 0:
        return [4, 5, 6, 7, 3, 2, 1, 0]
    def key(g):
        return (_cls(2 * g, u) == 0, g)
    return sorted(range(NG), key=key)


def build_program():
    nc = bacc.Bacc()

    xqT = nc.declare_dram_parameter("xqT", [D, S], f16, isOutput=False)
    xkT = nc.declare_dram_parameter("xkT", [D, S], f16, isOutput=False)
    xvT = nc.declare_dram_parameter("xvT", [D, S], f16, isOutput=False)
    wq = nc.declare_dram_parameter("wq", [128, 4, 256], f16, isOutput=False)
    wk = nc.declare_dram_parameter("wk", [128, 4, 256], f16, isOutput=False)
    wv = nc.declare_dram_parameter("wv", [128, 4, 256], f16, isOutput=False)
    wo = nc.declare_dram_parameter("wo", [64, 4, 512], f16, isOutput=False)
    ebtd = nc.declare_dram_parameter("ebt", [128, 4, 1408], f16, isOutput=False)
    cbd = nc.declare_dram_parameter("cb", [128, 4, 3], f32, isOutput=False)
    outd = nc.declare_dram_parameter("out", [S, D], f32, isOutput=True)

    with tile.TileContext(nc) as tc:
        with (
            tc.tile_pool(name="sb", bufs=1) as pool,
            tc.tile_pool(name="xt", bufs=1) as xpool,
            tc.tile_pool(name="pt", bufs=18) as ppool,
            tc.tile_pool(name="cxp", bufs=2) as cpool,
            tc.tile_pool(name="obp", bufs=4) as opool,
            tc.tile_pool(name="sc", bufs=2, space="PSUM") as scp,
            tc.tile_pool(name="cx", bufs=2, space="PSUM") as ctxpool,
            tc.tile_pool(name="ep", bufs=2, space="PSUM") as epp,
        ):
            # ---- persistent SBUF tiles -------------------------------------
            wq_sb = pool.tile([128, 4, 256], f16, tag="wq")
            wk_sb = pool.tile([128, 4, 256], f16, tag="wk")
            wv_sb = pool.tile([128, 4, 256], f16, tag="wv")
            wo_sb = pool.tile([64, 4, 512], f16, tag="wo")
            ebt_sb = pool.tile([128, 4, 1408], f16, tag="ebt")
            cb_sb = pool.tile([128, 4, 3], f32, tag="cb")
            qt_sb = pool.tile([128, 2, S], f16, tag="qt")
            kt_sb = pool.tile([128, 2, S], f16, tag="kt")
            v_sb = pool.tile([128, NKT, 4 * 65], f16, tag="v")
            ones_c = pool.tile([1, 64], f16, tag="ones")
            warm = pool.tile([128, 16], f32, tag="warm")

            # input tiles, consolidated so each is one DMA trigger:
            # xq0: q-unit 0 for all 4 d-blocks; xq123: q-units 1..3;
            # xka: k cols 1024:2048; xkbc: k cols 0:1024; xv01/xv23: V halves
            xq0 = xpool.tile([128, 4, 512], f16, tag="xq0")
            xq1 = xpool.tile([128, 4, 512], f16, tag="xq1")
            xq23 = xpool.tile([128, 4, 1024], f16, tag="xq23")
            xka1a = xpool.tile([128, 4, 256], f16, tag="xka1a")
            xka1b = xpool.tile([128, 4, 256], f16, tag="xka1b")
            xka2 = xpool.tile([128, 4, 512], f16, tag="xka2")
            xkbc = xpool.tile([128, 4, 1024], f16, tag="xkbc")
            xv01 = xpool.tile([128, 2, 2048], f16, tag="xv01")
            xv23 = xpool.tile([128, 2, 2048], f16, tag="xv23")

            def blk(t, cols):
                return t[:, cols].rearrange("(c p) s -> p c s", p=128)

            # ---- tier-1 DMA triggers: only what the first score groups
            # need.  Within a queue, packets of later DMAs interleave with
            # earlier ones (weak priority), so the bulk loads are *gated*
            # below rather than merely ordered after. --------------------
            nc.sync.dma_start(wq_sb[:], wq[:])
            nc.sync.dma_start(xq0[:], blk(xqT, slice(0, 512)))
            nc.gpsimd.dma_start(wk_sb[:], wk[:])
            nc.gpsimd.dma_start(cb_sb[:], cbd[:])
            nc.gpsimd.dma_start(xka1a[:], blk(xkT, slice(1024, 1280)))
            nc.gpsimd.dma_start(xka1b[:], blk(xkT, slice(1280, 1536)))
            nc.gpsimd.dma_start(xka2[:], blk(xkT, slice(1536, 2048)))

            # load the exp table set (one-time ~2.7us) while DMAs stream
            nc.vector.memset(warm[:], 0.0)
            nc.scalar.activation(warm[:], warm[:], AF.Exp, bias=0.0, scale=1.0)
            nc.vector.memset(ones_c[:], 1.0)

            # junk matmuls on the ones tile warm the PE's HAM clock gate up
            # while the first inputs stream in, so the first projections run
            # at full clock (sized to end roughly when the data lands)
            jw = epp.tile([128, 512], f32, tag="ep", name="jw")
            for _ in range(100):
                nc.tensor.matmul(jw[0:64, 0:64], lhsT=ones_c[:],
                                 rhs=ones_c[:], start=True, stop=True)

            def xv_slice(ct, cols):
                t = (xv01, xv23)[ct // 2]
                return t[:, ct % 2, cols]

            # ---- projection helpers ----------------------------------------
            def k_rhs(sc, ct):
                if sc < 2:
                    return xkbc[:, ct, sc * 512:(sc + 1) * 512]
                assert sc == 3
                return xka2[:, ct, :]

            def proj_k2_hp1():
                # head-pair-1 projection of k columns 1024:1536 (the input
                # for those columns lives in the two quarter tiles)
                pk = epp.tile([128, 512], f32, tag="ep", name="pk2h1")
                for qi, xt in enumerate((xka1a, xka1b)):
                    for ct in range(4):
                        nc.tensor.matmul(
                            pk[:, qi * 256:(qi + 1) * 256],
                            lhsT=wk_sb[:, ct, 128:256],
                            rhs=xt[:, ct, :],
                            start=(ct == 0), stop=(ct == 3),
                        )
                nc.vector.tensor_copy(kt_sb[:, 1, 1024:1536], pk[:])

            def proj_k_quarter(xt, cols):
                # 256-col K projection chunk (first score groups' columns)
                pk = epp.tile([128, 512], f32, tag="ep", name="pkq")
                for ct in range(4):
                    nc.tensor.matmul(
                        pk[:, 0:256],
                        lhsT=wk_sb[:, ct, 0:128],
                        rhs=xt[:, ct, :],
                        start=(ct == 0), stop=(ct == 3),
                    )
                nc.vector.tensor_copy(kt_sb[:, 0, cols], pk[:, 0:256])

            def q_rhs(sc, ct):
                if sc == 0:
                    return xq0[:, ct, :]
                if sc == 1:
                    return xq1[:, ct, :]
                return xq23[:, ct, (sc - 2) * 512:(sc - 1) * 512]

            def proj_group(w_sb, rhs_fn, dst, hp, sc, ptag):
                pk = (ctxpool if ptag == "cx" else epp).tile(
                    [128, 512], f32, tag=ptag, name=f"pj{hp}{sc}")
                for ct in range(4):
                    nc.tensor.matmul(
                        pk[:],
                        lhsT=w_sb[:, ct, hp * 128:(hp + 1) * 128],
                        rhs=rhs_fn(sc, ct),
                        start=(ct == 0), stop=(ct == 3),
                    )
                nc.vector.tensor_copy(dst[:, hp, sc * 512:(sc + 1) * 512], pk[:])

            # pre-loop projections: Q hp0 unit-0 and the K columns the first
            # two score groups need (sc3 follows as a pre-pop at gi2)
            proj_group(wq_sb, q_rhs, qt_sb, 0, 0, "cx")
            proj_k_quarter(xka1a, slice(1024, 1280))
            proj_k_quarter(xka1b, slice(1280, 1536))

            # ---- tier-2 DMA triggers, gated on the tier-1 critical path:
            # a tiny read joining the destination tile with kt_sb creates a
            # write-after-read dependency that holds each bulk DMA back
            # until the first projections' data has fully landed. ---------
            def gate(dst_tile):
                g = cpool.tile([1, 16], f32, tag="gate", bufs=8,
                               name="gate")
                nc.vector.tensor_add(g[:], dst_tile[0:1, 0, 0:16],
                                     kt_sb[0:1, 0, 1024:1040])
            for t in (wv_sb, xv01, xkbc, xq1, xq23, xv23, ebt_sb, wo_sb):
                gate(t)
            nc.sync.dma_start(xkbc[:], blk(xkT, slice(0, 1024)))
            nc.sync.dma_start(wv_sb[:], wv[:])
            nc.sync.dma_start(xv01[:],
                              xvT[0:256, :].rearrange("(c p) s -> p c s", p=128))
            nc.sync.dma_start(xq1[:], blk(xqT, slice(512, 1024)))
            nc.sync.dma_start(xq23[:], blk(xqT, slice(1024, 2048)))
            nc.gpsimd.dma_start(ebt_sb[:], ebtd[:])
            nc.gpsimd.dma_start(xv23[:],
                                xvT[256:512, :].rearrange("(c p) s -> p c s",
                                                          p=128))
            nc.gpsimd.dma_start(wo_sb[:], wo[:])

            def v_group(g):
                def emit():
                    pv = epp.tile([128, 512], f32, tag="ep", name=f"pv{g}")
                    for sti in range(2):
                        st = 2 * g + sti
                        for ct in range(4):
                            nc.tensor.matmul(
                                pv[:, sti * 256:sti * 256 + 256],
                                lhsT=xv_slice(ct, slice(st * 128, (st + 1) * 128)),
                                rhs=wv_sb[:, ct, :],
                                start=(ct == 0), stop=(ct == 3),
                            )
                    for sti in range(2):
                        st = 2 * g + sti
                        vslice = v_sb[:, st, :].rearrange(
                            "p (h x) -> p h x", x=65)
                        nc.vector.tensor_copy(
                            vslice[:, :, 0:64],
                            pv[:, sti * 256:sti * 256 + 256].rearrange(
                                "p (h x) -> p h x", x=64),
                        )
                        nc.vector.memset(vslice[:, :, 64:65], 1.0)
                return emit

            def pg_item(w_sb, rhs_fn, dst, hp, sc):
                def emit():
                    proj_group(w_sb, rhs_fn, dst, hp, sc, "ep")
                return emit

            def noop():
                pass

            # ---- attention + pipelined epilogue ----------------------------
            cx_tiles = {}     # (u, lh) -> normalized ctx [64, 512] f16
            ob_a = {}         # qs -> staged hp0 O-projection partial (f32)
            filler = []       # FIFO of emission closures
            avq = []          # globally deferred AV matmul emissions
            AV_LAG = 2

            def av_drain(n=None):
                k = len(avq) if n is None else n
                for _ in range(k):
                    if avq:
                        avq.pop(0)()

            def epilogue_items(u, hp, ctxps):
                """Normalization for the two heads of (u, hp), split into
                pipeline-friendly chunks.  The softmax denominators live on
                partition 64 (the AV ones-column row); the reciprocal and
                the broadcast matmul operate there directly (DVE lanes are
                per-partition, matmul row placement via tile_position), so
                no partition-moving DMA is needed."""
                state = {}

                def e1():
                    # l rows first, and the DMA trigger immediately after, so
                    # its engine-clock wait covers only these two copies
                    lr = cpool.tile([65, 1024], f32, tag="lr", bufs=2,
                                    name=f"lr{u}{hp}")
                    nc.vector.tensor_copy(lr[64:65, 0:512], ctxps[0][64:65, :])
                    nc.vector.tensor_copy(lr[64:65, 512:1024],
                                          ctxps[1][64:65, :])
                    lp = cpool.tile([1, 1024], f32, tag="lp", name=f"lp{u}{hp}")
                    nc.gpsimd.dma_start(lp[:], lr[64:65, :])
                    state["lp"] = lp

                def e2():
                    for ah in range(2):
                        ctxf = cpool.tile([64, 512], f32, tag="ctxf", bufs=4,
                                          name=f"ctxf{u}{hp}{ah}")
                        nc.vector.tensor_copy(ctxf[:], ctxps[ah][0:64, :])
                        state[ah] = ctxf

                def e2b():
                    linv = cpool.tile([1, 1024], f32, tag="linv",
                                      name=f"linv{u}{hp}")
                    nc.vector.reciprocal_approx_fast(linv[:], state["lp"][:])
                    linvb = cpool.tile([1, 1024], f16, tag="linvb",
                                       name=f"linvb{u}{hp}")
                    nc.vector.tensor_scalar_mul(linvb[:], linv[:], 256.0)
                    state["linvb"] = linvb

                def e3():
                    linvb = state["linvb"]
                    for ah in range(2):
                        bc = epp.tile([64, 512], f32, tag="ep",
                                      name=f"bc{u}{hp}{ah}")
                        nc.tensor.matmul(
                            bc[:], lhsT=ones_c[:],
                            rhs=linvb[:, ah * 512:(ah + 1) * 512],
                            start=True, stop=True)
                        cxn = cpool.tile([64, 512], f16, tag="cxn", bufs=8,
                                         name=f"cx{u}{hp}{ah}")
                        nc.vector.tensor_mul(cxn[:], bc[:], state[ah][:])
                        cx_tiles[(u, 2 * hp + ah)] = cxn

                return [e1, e2, e2b, e3]

            def oproj_items(u):
                items = []
                for qs in range(4):
                    def emit(u=u, qs=qs):
                        po = epp.tile([128, 512], f32, tag="ep",
                                      name=f"po{u}{qs}")
                        for lh in range(4):
                            nc.tensor.matmul(
                                po[:],
                                lhsT=cx_tiles[(u, lh)][:, qs * 128:(qs + 1) * 128],
                                rhs=wo_sb[:, lh, :],
                                start=(lh == 0), stop=(lh == 3),
                            )
                        ob = opool.tile([128, 512], f32, tag="ob",
                                        name=f"ob{u}{qs}")
                        nc.vector.tensor_copy(ob[:], po[:])
                        nc.sync.dma_start(
                            outd[u * 512 + qs * 128: u * 512 + (qs + 1) * 128, :],
                            ob[:],
                        )
                    items.append(emit)
                return items

            # split O-projection for the last u: stage the hp0-heads partial
            # during the last attention block, finish + combine in the tail
            def oproj_a_item(u, qs):
                def emit():
                    po = epp.tile([128, 512], f32, tag="ep",
                                  name=f"poa{u}{qs}")
                    for lh in range(2):
                        nc.tensor.matmul(
                            po[:],
                            lhsT=cx_tiles[(u, lh)][:, qs * 128:(qs + 1) * 128],
                            rhs=wo_sb[:, lh, :],
                            start=(lh == 0), stop=(lh == 1),
                        )
                    oba = opool.tile([128, 512], f32, tag="oba", bufs=4,
                                     name=f"oba{u}{qs}")
                    nc.vector.tensor_copy(oba[:], po[:])
                    ob_a[qs] = oba
                return emit

            def oproj_b_item(u, qs):
                def emit():
                    po = epp.tile([128, 512], f32, tag="ep",
                                  name=f"pob{u}{qs}")
                    for lh in range(2, 4):
                        nc.tensor.matmul(
                            po[:],
                            lhsT=cx_tiles[(u, lh)][:, qs * 128:(qs + 1) * 128],
                            rhs=wo_sb[:, lh, :],
                            start=(lh == 2), stop=(lh == 3),
                        )
                    ob = opool.tile([128, 512], f32, tag="ob",
                                    name=f"ob{u}{qs}")
                    nc.vector.tensor_add(ob[:], po[:], ob_a[qs][:])
                    nc.sync.dma_start(
                        outd[u * 512 + qs * 128: u * 512 + (qs + 1) * 128, :],
                        ob[:],
                    )
                return emit

            def keep_warm(n=8):
                # junk matmuls bridging the tail normalization chain so HAM
                # doesn't re-throttle the PE before the final O-projection
                dz = epp.tile([128, 512], f32, tag="ep", name="dz")
                for r in range(n):
                    vsl = v_sb[:, r, :].rearrange(
                        "p (h x) -> p h x", x=65)[:, 0, :]
                    nc.tensor.matmul(dz[0:65, :], lhsT=vsl,
                                     rhs=qt_sb[:, 0, 0:512],
                                     start=True, stop=True)

            def attention(u, hp, pre_sched=None, post_sched=None, post_rate=1,
                          av_lag=None):
                """pre_sched/post_sched: {gi: [closures]} emitted before the
                scores (pre) or between the exps and the AV matmuls (post) of
                group gi.  post_rate: queued filler items popped at each post
                point.  AV matmuls are appended to the global deferred queue
                and drained AV_LAG groups later (carrying across blocks)."""
                ctxps = [
                    ctxpool.tile([65, 512], f32, tag="cx",
                                 name=f"ctxp{u}{hp}{i}")
                    for i in range(2)
                ]
                nav = [0, 0]
                carry = {"n": len(avq)}

                def av_item(g, srcs):
                    def emit():
                        for ah in range(2):
                            lh = 2 * hp + ah
                            for ti in range(2):
                                t = 2 * g + ti
                                vsl = v_sb[:, t, :].rearrange(
                                    "p (h x) -> p h x", x=65)[:, lh, :]
                                nav[ah] += 1
                                nc.tensor.matmul(
                                    ctxps[ah][:],
                                    lhsT=vsl,
                                    rhs=srcs[ah][:, ti * 512:(ti + 1) * 512],
                                    start=(nav[ah] == 1),
                                    stop=(nav[ah] == NKT),
                                )
                    return emit

                for gi, g in enumerate(_gorder(u)):
                    for fn in (pre_sched or {}).get(gi, []):
                        fn()
                    cls = _cls(2 * g, u)
                    sct = [scp.tile([128, 1024], f32, tag="sc",
                                    name=f"sct{u}{hp}{g}{i}")
                           for i in range(2)]
                    # ti-major: the two quadrant (ah) matmuls of each ti run
                    # concurrently in the PE array
                    for ti in range(2):
                        t = 2 * g + ti
                        for ah in range(2):
                            nc.tensor.matmul(
                                sct[ah][:, ti * 512:(ti + 1) * 512],
                                lhsT=kt_sb[ah * 64:(ah + 1) * 64, hp,
                                           t * 128:(t + 1) * 128],
                                rhs=qt_sb[ah * 64:(ah + 1) * 64, hp,
                                          u * 512:(u + 1) * 512],
                                start=True, stop=True,
                                tile_position=(ah * 64, 0),
                            )
                    srcs = []
                    for ah in range(2):
                        lh = 2 * hp + ah
                        pt = ppool.tile([128, 1024], f16, tag="pt",
                                        name=f"pt{u}{hp}{g}{ah}")
                        nc.scalar.activation(
                            pt[:], sct[ah][:], AF.Exp,
                            bias=cb_sb[:, lh, cls:cls + 1], scale=1.0,
                        )
                        srcs.append(pt)
                    for fn in (post_sched or {}).get(gi, []):
                        fn()
                    # the previous block's carried AV matmuls must all be
                    # emitted before its epilogue (a filler item) can be:
                    # dependency tracking only covers emitted writers.  They
                    # drain at most 3 per group so the score stream is never
                    # delayed by a long AV clump.
                    if carry["n"]:
                        k = min(3, carry["n"])
                        carry["n"] -= k
                        av_drain(k)
                    for _ in range(post_rate):
                        if filler:
                            filler.pop(0)()
                    if cls == 0:
                        for ah in range(2):
                            lh = 2 * hp + ah
                            pt = srcs[ah]
                            src = ppool.tile([128, 1024], f16, tag="src",
                                             bufs=8,
                                             name=f"src{u}{hp}{g}{ah}")
                            for ti in range(2):
                                col = _ebt_col(2 * g + ti, u)
                                nc.vector.tensor_mul(
                                    src[:, ti * 512:(ti + 1) * 512],
                                    pt[:, ti * 512:(ti + 1) * 512],
                                    ebt_sb[:, lh, col:col + 512],
                                )
                            srcs[ah] = src
                    avq.append(av_item(g, srcs))
                    if not carry["n"]:
                        while len(avq) > (AV_LAG if av_lag is None else av_lag):
                            avq.pop(0)()
                return ctxps

            # ---- block (0,0): explicit schedules ---------------------------
            # gorder(0) = [4,5,6,7,3,2,1,0]
            pre00 = {
                2: [pg_item(wk_sb, k_rhs, kt_sb, 0, 3)],
                4: [pg_item(wk_sb, k_rhs, kt_sb, 0, 1)],
                6: [pg_item(wk_sb, k_rhs, kt_sb, 0, 0)],
            }
            post00 = {
                0: [lambda: keep_warm(4)],
                1: [lambda: keep_warm(4)],
                2: [lambda: keep_warm(4)],
                3: [lambda: keep_warm(4)],
                5: [lambda: keep_warm(4)],
                6: [v_group(4), v_group(5),
                    pg_item(wk_sb, k_rhs, kt_sb, 1, 0),
                    pg_item(wk_sb, k_rhs, kt_sb, 1, 1)],
                7: [v_group(6), v_group(7), v_group(3),
                    proj_k2_hp1,
                    pg_item(wk_sb, k_rhs, kt_sb, 1, 3)],
            }
            ctxps = attention(0, 0, pre00, post00, post_rate=0, av_lag=8)
            filler.extend([pg_item(wq_sb, q_rhs, qt_sb, 1, 1),
                           pg_item(wq_sb, q_rhs, qt_sb, 0, 1),
                           pg_item(wq_sb, q_rhs, qt_sb, 1, 2),
                           pg_item(wq_sb, q_rhs, qt_sb, 1, 3),
                           pg_item(wq_sb, q_rhs, qt_sb, 0, 2),
                           pg_item(wq_sb, q_rhs, qt_sb, 0, 3)])
            filler.extend(epilogue_items(0, 0, ctxps))

            # hp1 unit-0 Q projection must precede block (0,1)'s scores
            pre01 = {0: [pg_item(wq_sb, q_rhs, qt_sb, 1, 0)]}
            post01 = {
                0: [v_group(2), v_group(1), v_group(0)],
            }
            ctxps = attention(0, 1, pre01, post01, post_rate=2)
            filler.extend(epilogue_items(0, 1, ctxps))
            filler.extend(oproj_items(0))

            for u in range(1, NU):
                for hp in range(2):
                    last = (u == NU - 1 and hp == 1)
                    ctxps = attention(u, hp, post_rate=2 if last else 1)
                    filler.extend(epilogue_items(u, hp, ctxps))
                    if hp == 1 and not last:
                        filler.extend(oproj_items(u))
                    if u == NU - 1 and hp == 0:
                        # stage the hp0-heads O-projection partials inside
                        # the last attention block
                        filler.extend(oproj_a_item(u, qs) for qs in range(4))
            # tail: drain deferred AVs, keep the PE warm through the final
            # normalization chain, then finish the split O-projection
            av_drain()
            tail = list(filler)
            filler.clear()
            keep_warm(6)
            for i, fn in enumerate(tail):
                fn()
                if i in (0, 2):
                    keep_warm(6)
            for qs in range(4):
                oproj_b_item(NU - 1, qs)()

    nc.compile()
    return nc


_PROGRAM = None


def _get_program():
    global _PROGRAM
    if _PROGRAM is None:
        _PROGRAM = build_program()
    return _PROGRAM


def kernel(**inputs):
    query = np.asarray(inputs["query"], dtype=np.float32)
    key = np.asarray(inputs["key"], dtype=np.float32)
    value = np.asarray(inputs["value"], dtype=np.float32)
    mask = np.asarray(inputs["mask"])
    Wq = np.asarray(inputs["Wq"], dtype=np.float32)
    Wk = np.asarray(inputs["Wk"], dtype=np.float32)
    Wv = np.asarray(inputs["Wv"], dtype=np.float32)
    Wo = np.asarray(inputs["Wo"], dtype=np.float32)
    bo = np.asarray(inputs["bo"], dtype=np.float32)
    rel_bias = np.asarray(inputs["rel_bias"], dtype=np.float32)

    if not np.all(mask != 0):
        raise NotImplementedError("kernel assumes an all-ones attention mask")

    nc = _get_program()
    scale = np.float32(1.0 / np.sqrt(DK))

    # sliding-window exp-bias table: ebt[p, lh, j] = exp(tbl[clip(895-j+p)])
    pp = np.arange(128)[:, None]
    jj = np.arange(1408)[None, :]
    widx = np.clip(895 - jj + pp, 0, 510)  # [128, 1408]

    in_maps = []
    for c in range(NCORES):
        b = c // 2
        hbase = (c % 2) * 4
        rows = slice(hbase * 64, (hbase + 4) * 64)

        wq_arr = np.ascontiguousarray(
            (Wq[rows, :] * scale).T.reshape(4, 128, 256).swapaxes(0, 1))
        wk_arr = np.ascontiguousarray(
            Wk[rows, :].T.reshape(4, 128, 256).swapaxes(0, 1))
        wv_arr = np.ascontiguousarray(
            Wv[rows, :].T.reshape(4, 128, 256).swapaxes(0, 1))

        wo_arr = np.empty((64, 4, 512), dtype=np.float32)
        ebt_arr = np.empty((128, 4, 1408), dtype=np.float16)
        cb_arr = np.zeros((128, 4, 3), dtype=np.float32)
        for lh in range(4):
            g = hbase + lh
            wo_arr[:, lh, :] = Wo[:, g * 64:(g + 1) * 64].T * (1.0 / 256.0)
            tbl = rel_bias[g]
            ebt_arr[:, lh, :] = np.exp(tbl)[widx]
            cb_arr[:, lh, 1] = tbl[0]
            cb_arr[:, lh, 2] = tbl[510]

        bf = np.float16
        in_maps.append({
            "xqT": np.ascontiguousarray(query[b].T).astype(bf),
            "xkT": np.ascontiguousarray(key[b].T).astype(bf),
            "xvT": np.ascontiguousarray(value[b].T).astype(bf),
            "wq": wq_arr.astype(bf), "wk": wk_arr.astype(bf),
            "wv": wv_arr.astype(bf), "wo": wo_arr.astype(bf),
            "ebt": ebt_arr, "cb": cb_arr,
        })

    res = run_bass_kernel_spmd(nc, in_maps, list(range(NCORES)), trace=False)

    out = np.zeros((B, S, D), dtype=np.float32)
    for c in range(NCORES):
        out[c // 2] += res.results[c]["out"]
    out += bo[None, None, :]
    return out


# revision 44
# speedup vs baseline: 1.0564x; 1.0564x over previous
"""Trainium2 Bass kernel for nn_MultiHeadAttention_34162169872901.

MultiHeadAttention (B=4, S=2048, d_model=512, 8 heads, d_k=64) with a
relative-position bias table (511 entries, clamp +-255) and an all-ones mask.

Sharding (8 NeuronCores): core c handles batch b = c//2 and 4 of the 8 heads
(c%2 selects the head half) -- data parallel on B, tensor parallel on heads.
Each core computes its 4 heads' Q/K/V projections, the full attention for its
batch, normalization, and its partial output projection; the host sums the two
partial outputs per batch (and adds the output bias bo).

v4 -- scheduled for the ACT (exp) bottleneck:
  - The softmax exp on the scalar/ACT engine is the binding resource
    (128 x [128,1024] exps ~= 1.13us each).  The schedule starts exp as
    early as possible and tries to never let ACT starve.
  - DMA triggers cost ~650ns each and serialize per engine, so inputs are
    consolidated into a few multi-block tiles, triggered first thing in
    the program, in need order, on the two fast queues (sync + gpsimd).
  - Score matmuls are emitted ah-major so the quadrant whose PSUM bank
    frees first is refilled without waiting for the second exp.
  - AV matmuls are globally deferred by two groups (and carry across
    block boundaries) so filler never blocks the score stream and the
    next block's first scores precede the previous block's AV drain.
  - The relative-bias exp-table is a [128, 4, 1408] sliding-window table
    (every in-band Toeplitz block is a contiguous 512-col slice).
  - PSUM: scores ring 2x[128,1024] (4 banks), AV-accumulator ring
    2x[65,512] (2 banks), epilogue/projection ring 2x[128,512] (2 banks).
  - Per-(u,hp) normalization and the O-projection are pipelined into the
    following attention block; the final O-projection is split so half is
    staged during the last block, with keep-warm matmuls bridging the
    final normalization chain.
"""

import sys
import types

import numpy as np

B = 4
S = 2048
D = 512
NHEAD = 8
DK = 64
NCORES = 8
MAX_REL = 255
NKT = S // 128   # 16 k-tiles
NU = S // 512    # 4 q-units
NG = NKT // 2    # 8 score groups per (u, hp)


def _install_axon_hooks():
    """Provide antenv.axon_hooks (missing in this image) so bass_utils'
    trace path can be used; harmless when tracing is off."""
    try:
        import antenv
    except ImportError:
        return
    try:
        from antenv.axon_hooks import get_axon_ntff_profile_hook  # noqa: F401
        return
    except ImportError:
        pass
    hook = None
    try:
        from trn_agent_boot.trn_boot import _ntff_profile_via_ctypes
        hook = _ntff_profile_via_ctypes("/opt/axon/libaxon_pjrt.so")
    except Exception:
        hook = None
    m = types.ModuleType("antenv.axon_hooks")
    m.get_axon_ntff_profile_hook = lambda: hook
    m.set_axon_ntff_profile_hook = lambda h: None
    sys.modules["antenv.axon_hooks"] = m
    antenv.axon_hooks = m


_install_axon_hooks()

import concourse.bass as bass  # noqa: E402
import concourse.bacc as bacc  # noqa: E402
import concourse.mybir as mybir  # noqa: E402
from concourse import tile  # noqa: E402
from concourse.bass_utils import run_bass_kernel_spmd  # noqa: E402
from concourse.vector_clock import ScopedClock as _ScopedClock  # noqa: E402

f32 = mybir.dt.float32
bf16 = mybir.dt.bfloat16
f16 = mybir.dt.float16
AF = mybir.ActivationFunctionType


def _patched_drain_and_barrier(self, tick_clock, wait_clock):
    # walrus in this container rejects >2 sem waits on one instruction; emit
    # the tail-drain waits as standalone wait instructions instead.
    nc = self.nc
    dummy = mybir.InstNoOp(name="drain-wait-probe", engine=mybir.EngineType.SP)
    wait_clock.add_sem_waits(dummy, _ScopedClock({None: tick_clock.global_clock}))
    handles = {h.name: h for h in self.sems.allocated().values()}
    si = dummy.sync_info
    for w in (si.on_wait if si is not None else []):
        nc.sync.wait_ge(handles[w.ant_name], w.wait_value)
    nc.sync.drain()
    nc.all_engine_barrier()
    popped = nc._tile_sem_poison_stack.pop()
    assert popped is self._sem_poison
    nc.clear_and_free_semaphores(list(self.sems.allocated().values()))
    nc.all_engine_barrier()


tile.TileContext._drain_and_barrier = _patched_drain_and_barrier


def _delta(t, u):
    # key-tile offset minus query-chunk offset; bias entry index is
    # delta + (p - f) + 255 clipped to [0, 510]
    return 128 * t - 512 * u


def _cls(t, u):
    d = _delta(t, u)
    if d <= -384:
        return 1  # whole block clamps to table[0]
    if d >= 768:
        return 2  # whole block clamps to table[510]
    return 0      # in-band: needs the Toeplitz block


def _ebt_col(t, u):
    # start column of the [128,512] in-band block inside the 1408-wide
    # sliding-window exp-bias table: 640 - delta
    return 640 - _delta(t, u)


def _gorder(u):
    # out-of-band groups first (no bias-table dependency, constant folded
    # into the exp bias).  For u == 0, order by K-projection availability:
    # high k-columns first, in-band groups in descending g.
    if u == 0:
        return [4, 5, 6, 7, 3, 2, 1, 0]
    def key(g):
        return (_cls(2 * g, u) == 0, g)
    return sorted(range(NG), key=key)


def build_program():
    nc = bacc.Bacc()

    xqT = nc.declare_dram_parameter("xqT", [D, S], f16, isOutput=False)
    xkT = nc.declare_dram_parameter("xkT", [D, S], f16, isOutput=False)
    xvT = nc.declare_dram_parameter("xvT", [D, S], f16, isOutput=False)
    wq = nc.declare_dram_parameter("wq", [128, 4, 256], f16, isOutput=False)
    wk = nc.declare_dram_parameter("wk", [128, 4, 256], f16, isOutput=False)
    wv = nc.declare_dram_parameter("wv", [128, 4, 256], f16, isOutput=False)
    wo = nc.declare_dram_parameter("wo", [64, 4, 512], f16, isOutput=False)
    ebtd = nc.declare_dram_parameter("ebt", [128, 4, 1408], f16, isOutput=False)
    cbd = nc.declare_dram_parameter("cb", [128, 4, 3], f32, isOutput=False)
    outd = nc.declare_dram_parameter("out", [S, D], f32, isOutput=True)

    with tile.TileContext(nc) as tc:
        with (
            tc.tile_pool(name="sb", bufs=1) as pool,
            tc.tile_pool(name="xt", bufs=1) as xpool,
            tc.tile_pool(name="pt", bufs=14) as ppool,
            tc.tile_pool(name="cxp", bufs=2) as cpool,
            tc.tile_pool(name="obp", bufs=4) as opool,
            tc.tile_pool(name="sc", bufs=2, space="PSUM") as scp,
            tc.tile_pool(name="cx", bufs=2, space="PSUM") as ctxpool,
            tc.tile_pool(name="ep", bufs=2, space="PSUM") as epp,
        ):
            # ---- persistent SBUF tiles -------------------------------------
            wq_sb = pool.tile([128, 4, 256], f16, tag="wq")
            wk_sb = pool.tile([128, 4, 256], f16, tag="wk")
            wv_sb = pool.tile([128, 4, 256], f16, tag="wv")
            wo_sb = pool.tile([64, 4, 512], f16, tag="wo")
            ebt_sb = pool.tile([128, 4, 1408], f16, tag="ebt")
            cb_sb = pool.tile([128, 4, 3], f32, tag="cb")
            qt_sb = pool.tile([128, 2, S], f16, tag="qt")
            kt_sb = pool.tile([128, 2, S], f16, tag="kt")
            v_sb = pool.tile([128, NKT, 4 * 65], f16, tag="v")
            ones_c = pool.tile([1, 64], f16, tag="ones")
            warm = pool.tile([128, 16], f32, tag="warm")

            # input tiles, consolidated so each is one DMA trigger:
            # xq0: q-unit 0 for all 4 d-blocks; xq123: q-units 1..3;
            # xka: k cols 1024:2048; xkbc: k cols 0:1024; xv01/xv23: V halves
            xq0 = xpool.tile([128, 4, 512], f16, tag="xq0")
            xq123 = xpool.tile([128, 4, 1536], f16, tag="xq123")
            xka = xpool.tile([128, 4, 1024], f16, tag="xka")
            xkbc = xpool.tile([128, 4, 1024], f16, tag="xkbc")
            xv01 = xpool.tile([128, 2, 2048], f16, tag="xv01")
            xv23 = xpool.tile([128, 2, 2048], f16, tag="xv23")

            def blk(t, cols):
                return t[:, cols].rearrange("(c p) s -> p c s", p=128)

            # ---- DMA triggers first (each costs ~650ns of engine time and
            # the data cannot start moving until its trigger runs) ----------
            nc.sync.dma_start(wq_sb[:], wq[:])
            nc.sync.dma_start(wv_sb[:], wv[:])
            nc.sync.dma_start(xq0[:], blk(xqT, slice(0, 512)))
            nc.sync.dma_start(xv01[:],
                              xvT[0:256, :].rearrange("(c p) s -> p c s", p=128))
            nc.sync.dma_start(xkbc[:], blk(xkT, slice(0, 1024)))
            nc.sync.dma_start(xq123[:], blk(xqT, slice(512, 2048)))
            nc.sync.dma_start(wo_sb[:], wo[:])
            nc.gpsimd.dma_start(wk_sb[:], wk[:])
            nc.gpsimd.dma_start(cb_sb[:], cbd[:])
            nc.gpsimd.dma_start(xka[:], blk(xkT, slice(1024, 2048)))
            nc.gpsimd.dma_start(xv23[:],
                                xvT[256:512, :].rearrange("(c p) s -> p c s",
                                                          p=128))
            nc.gpsimd.dma_start(ebt_sb[:], ebtd[:])

            # load the exp table set (one-time ~2.7us) while DMAs stream
            nc.vector.memset(warm[:], 0.0)
            nc.scalar.activation(warm[:], warm[:], AF.Exp, bias=0.0, scale=1.0)
            nc.vector.memset(ones_c[:], 1.0)

            def xv_slice(ct, cols):
                t = (xv01, xv23)[ct // 2]
                return t[:, ct % 2, cols]

            # ---- projection helpers ----------------------------------------
            def k_rhs(sc, ct):
                if sc < 2:
                    return xkbc[:, ct, sc * 512:(sc + 1) * 512]
                return xka[:, ct, (sc - 2) * 512:(sc - 1) * 512]

            def q_rhs(sc, ct):
                if sc == 0:
                    return xq0[:, ct, :]
                return xq123[:, ct, (sc - 1) * 512:sc * 512]

            def proj_group(w_sb, rhs_fn, dst, hp, sc, ptag):
                pk = (ctxpool if ptag == "cx" else epp).tile(
                    [128, 512], f32, tag=ptag, name=f"pj{hp}{sc}")
                for ct in range(4):
                    nc.tensor.matmul(
                        pk[:],
                        lhsT=w_sb[:, ct, hp * 128:(hp + 1) * 128],
                        rhs=rhs_fn(sc, ct),
                        start=(ct == 0), stop=(ct == 3),
                    )
                nc.vector.tensor_copy(dst[:, hp, sc * 512:(sc + 1) * 512], pk[:])

            # pre-loop projections: Q hp0 unit-0 and K hp0 high columns
            proj_group(wq_sb, q_rhs, qt_sb, 0, 0, "ep")
            proj_group(wk_sb, k_rhs, kt_sb, 0, 2, "cx")
            proj_group(wk_sb, k_rhs, kt_sb, 0, 3, "cx")

            def v_group(g):
                def emit():
                    pv = epp.tile([128, 512], f32, tag="ep", name=f"pv{g}")
                    for sti in range(2):
                        st = 2 * g + sti
                        for ct in range(4):
                            nc.tensor.matmul(
                                pv[:, sti * 256:sti * 256 + 256],
                                lhsT=xv_slice(ct, slice(st * 128, (st + 1) * 128)),
                                rhs=wv_sb[:, ct, :],
                                start=(ct == 0), stop=(ct == 3),
                            )
                    for sti in range(2):
                        st = 2 * g + sti
                        vslice = v_sb[:, st, :].rearrange(
                            "p (h x) -> p h x", x=65)
                        nc.vector.tensor_copy(
                            vslice[:, :, 0:64],
                            pv[:, sti * 256:sti * 256 + 256].rearrange(
                                "p (h x) -> p h x", x=64),
                        )
                        nc.vector.memset(vslice[:, :, 64:65], 1.0)
                return emit

            def pg_item(w_sb, rhs_fn, dst, hp, sc):
                def emit():
                    proj_group(w_sb, rhs_fn, dst, hp, sc, "ep")
                return emit

            def noop():
                pass

            # ---- attention + pipelined epilogue ----------------------------
            cx_tiles = {}     # (u, lh) -> normalized ctx [64, 512] f16
            ob_a = {}         # qs -> staged hp0 O-projection partial (f32)
            filler = []       # FIFO of emission closures
            avq = []          # globally deferred AV matmul emissions
            AV_LAG = 2

            def av_drain(n=None):
                k = len(avq) if n is None else n
                for _ in range(k):
                    if avq:
                        avq.pop(0)()

            def epilogue_items(u, hp, ctxps):
                """Normalization for the two heads of (u, hp), split into
                pipeline-friendly chunks (with no-op spacers so the serial
                chain never blocks an engine queue)."""
                state = {}

                def e1():
                    for ah in range(2):
                        ctxf = cpool.tile([65, 512], f32, tag="ctxf", bufs=4,
                                          name=f"ctxf{u}{hp}{ah}")
                        nc.vector.tensor_copy(ctxf[:], ctxps[ah][:])
                        state[ah] = ctxf

                def e2a():
                    lp = cpool.tile([1, 1024], f32, tag="lp", name=f"lp{u}{hp}")
                    nc.gpsimd.dma_start(lp[:, 0:512], state[0][64:65, :])
                    nc.gpsimd.dma_start(lp[:, 512:1024], state[1][64:65, :])
                    state["lp"] = lp

                def e2b():
                    linv = cpool.tile([1, 1024], f32, tag="linv",
                                      name=f"linv{u}{hp}")
                    nc.vector.reciprocal_approx_fast(linv[:], state["lp"][:])
                    linvb = cpool.tile([1, 1024], f16, tag="linvb",
                                       name=f"linvb{u}{hp}")
                    nc.vector.tensor_scalar_mul(linvb[:], linv[:], 256.0)
                    state["linvb"] = linvb

                def e3():
                    linvb = state["linvb"]
                    for ah in range(2):
                        bc = epp.tile([64, 512], f32, tag="ep",
                                      name=f"bc{u}{hp}{ah}")
                        nc.tensor.matmul(
                            bc[:], lhsT=ones_c[:],
                            rhs=linvb[:, ah * 512:(ah + 1) * 512],
                            start=True, stop=True)
                        cxn = cpool.tile([64, 512], f16, tag="cxn", bufs=8,
                                         name=f"cx{u}{hp}{ah}")
                        nc.vector.tensor_mul(cxn[:], bc[:], state[ah][0:64, :])
                        cx_tiles[(u, 2 * hp + ah)] = cxn

                return [e1, e2a, noop, e2b, noop, e3]

            def oproj_items(u):
                items = []
                for qs in range(4):
                    def emit(u=u, qs=qs):
                        po = epp.tile([128, 512], f32, tag="ep",
                                      name=f"po{u}{qs}")
                        for lh in range(4):
                            nc.tensor.matmul(
                                po[:],
                                lhsT=cx_tiles[(u, lh)][:, qs * 128:(qs + 1) * 128],
                                rhs=wo_sb[:, lh, :],
                                start=(lh == 0), stop=(lh == 3),
                            )
                        ob = opool.tile([128, 512], f32, tag="ob",
                                        name=f"ob{u}{qs}")
                        nc.vector.tensor_copy(ob[:], po[:])
                        nc.sync.dma_start(
                            outd[u * 512 + qs * 128: u * 512 + (qs + 1) * 128, :],
                            ob[:],
                        )
                    items.append(emit)
                return items

            # split O-projection for the last u: stage the hp0-heads partial
            # during the last attention block, finish + combine in the tail
            def oproj_a_item(u, qs):
                def emit():
                    po = epp.tile([128, 512], f32, tag="ep",
                                  name=f"poa{u}{qs}")
                    for lh in range(2):
                        nc.tensor.matmul(
                            po[:],
                            lhsT=cx_tiles[(u, lh)][:, qs * 128:(qs + 1) * 128],
                            rhs=wo_sb[:, lh, :],
                            start=(lh == 0), stop=(lh == 1),
                        )
                    oba = opool.tile([128, 512], f32, tag="oba", bufs=4,
                                     name=f"oba{u}{qs}")
                    nc.vector.tensor_copy(oba[:], po[:])
                    ob_a[qs] = oba
                return emit

            def oproj_b_item(u, qs):
                def emit():
                    po = epp.tile([128, 512], f32, tag="ep",
                                  name=f"pob{u}{qs}")
                    for lh in range(2, 4):
                        nc.tensor.matmul(
                            po[:],
                            lhsT=cx_tiles[(u, lh)][:, qs * 128:(qs + 1) * 128],
                            rhs=wo_sb[:, lh, :],
                            start=(lh == 2), stop=(lh == 3),
                        )
                    ob = opool.tile([128, 512], f32, tag="ob",
                                    name=f"ob{u}{qs}")
                    nc.vector.tensor_add(ob[:], po[:], ob_a[qs][:])
                    nc.sync.dma_start(
                        outd[u * 512 + qs * 128: u * 512 + (qs + 1) * 128, :],
                        ob[:],
                    )
                return emit

            def keep_warm(n=8):
                # junk matmuls bridging the tail normalization chain so HAM
                # doesn't re-throttle the PE before the final O-projection
                dz = ctxpool.tile([65, 512], f32, tag="cx", name="dz")
                for r in range(n):
                    vsl = v_sb[:, r, :].rearrange(
                        "p (h x) -> p h x", x=65)[:, 0, :]
                    nc.tensor.matmul(dz[:], lhsT=vsl, rhs=qt_sb[:, 0, 0:512],
                                     start=True, stop=True)

            def attention(u, hp, pre_sched=None, post_sched=None, post_rate=1):
                """pre_sched/post_sched: {gi: [closures]} emitted before the
                scores (pre) or between the exps and the AV matmuls (post) of
                group gi.  post_rate: queued filler items popped at each post
                point.  AV matmuls are appended to the global deferred queue
                and drained AV_LAG groups later (carrying across blocks)."""
                ctxps = [
                    ctxpool.tile([65, 512], f32, tag="cx",
                                 name=f"ctxp{u}{hp}{i}")
                    for i in range(2)
                ]
                nav = [0, 0]

                def av_item(g, srcs):
                    def emit():
                        for ah in range(2):
                            lh = 2 * hp + ah
                            for ti in range(2):
                                t = 2 * g + ti
                                vsl = v_sb[:, t, :].rearrange(
                                    "p (h x) -> p h x", x=65)[:, lh, :]
                                nav[ah] += 1
                                nc.tensor.matmul(
                                    ctxps[ah][:],
                                    lhsT=vsl,
                                    rhs=srcs[ah][:, ti * 512:(ti + 1) * 512],
                                    start=(nav[ah] == 1),
                                    stop=(nav[ah] == NKT),
                                )
                    return emit

                for gi, g in enumerate(_gorder(u)):
                    for fn in (pre_sched or {}).get(gi, []):
                        fn()
                    cls = _cls(2 * g, u)
                    sct = [scp.tile([128, 1024], f32, tag="sc",
                                    name=f"sct{u}{hp}{g}{i}")
                           for i in range(2)]
                    # ah-major: refill the freed PSUM bank's quadrant first
                    for ah in range(2):
                        for ti in range(2):
                            t = 2 * g + ti
                            nc.tensor.matmul(
                                sct[ah][:, ti * 512:(ti + 1) * 512],
                                lhsT=kt_sb[ah * 64:(ah + 1) * 64, hp,
                                           t * 128:(t + 1) * 128],
                                rhs=qt_sb[ah * 64:(ah + 1) * 64, hp,
                                          u * 512:(u + 1) * 512],
                                start=True, stop=True,
                                tile_position=(ah * 64, 0),
                            )
                    srcs = []
                    for ah in range(2):
                        lh = 2 * hp + ah
                        pt = ppool.tile([128, 1024], f16, tag="pt",
                                        name=f"pt{u}{hp}{g}{ah}")
                        nc.scalar.activation(
                            pt[:], sct[ah][:], AF.Exp,
                            bias=cb_sb[:, lh, cls:cls + 1], scale=1.0,
                        )
                        srcs.append(pt)
                    if gi == 0:
                        # the previous block's carried AV matmuls must all be
                        # emitted before its epilogue (a filler item below)
                        # can be: dependency tracking only covers emitted
                        # writers
                        av_drain()
                    for fn in (post_sched or {}).get(gi, []):
                        fn()
                    for _ in range(post_rate):
                        if filler:
                            filler.pop(0)()
                    if cls == 0:
                        for ah in range(2):
                            lh = 2 * hp + ah
                            pt = srcs[ah]
                            src = ppool.tile([128, 1024], f16, tag="src",
                                             bufs=8,
                                             name=f"src{u}{hp}{g}{ah}")
                            for ti in range(2):
                                col = _ebt_col(2 * g + ti, u)
                                nc.vector.tensor_mul(
                                    src[:, ti * 512:(ti + 1) * 512],
                                    pt[:, ti * 512:(ti + 1) * 512],
                                    ebt_sb[:, lh, col:col + 512],
                                )
                            srcs[ah] = src
                    avq.append(av_item(g, srcs))
                    while len(avq) > AV_LAG:
                        avq.pop(0)()
                return ctxps

            # ---- block (0,0): explicit schedules ---------------------------
            # gorder(0) = [4,5,6,7,3,2,1,0]
            pre00 = {
                4: [pg_item(wk_sb, k_rhs, kt_sb, 0, 1)],
                6: [pg_item(wk_sb, k_rhs, kt_sb, 0, 0)],
            }
            post00 = {
                1: [v_group(4)],
                2: [v_group(5), v_group(6)],
                3: [v_group(7), v_group(3)],
                4: [v_group(2)],
                5: [v_group(1),
                    pg_item(wk_sb, k_rhs, kt_sb, 1, 0),
                    pg_item(wk_sb, k_rhs, kt_sb, 1, 1)],
                6: [v_group(0),
                    pg_item(wk_sb, k_rhs, kt_sb, 1, 2),
                    pg_item(wk_sb, k_rhs, kt_sb, 1, 3)],
            }
            ctxps = attention(0, 0, pre00, post00, post_rate=0)
            filler.extend([pg_item(wq_sb, q_rhs, qt_sb, 1, 1),
                           pg_item(wq_sb, q_rhs, qt_sb, 1, 2),
                           pg_item(wq_sb, q_rhs, qt_sb, 1, 3),
                           pg_item(wq_sb, q_rhs, qt_sb, 0, 1),
                           pg_item(wq_sb, q_rhs, qt_sb, 0, 2),
                           pg_item(wq_sb, q_rhs, qt_sb, 0, 3)])
            filler.extend(epilogue_items(0, 0, ctxps))

            # hp1 unit-0 Q projection must precede block (0,1)'s scores
            pre01 = {0: [pg_item(wq_sb, q_rhs, qt_sb, 1, 0)]}
            ctxps = attention(0, 1, pre01, None, post_rate=2)
            filler.extend(epilogue_items(0, 1, ctxps))
            filler.extend(oproj_items(0))

            for u in range(1, NU):
                for hp in range(2):
                    last = (u == NU - 1 and hp == 1)
                    ctxps = attention(u, hp, post_rate=2 if last else 1)
                    filler.extend(epilogue_items(u, hp, ctxps))
                    if hp == 1 and not last:
                        filler.extend(oproj_items(u))
                    if u == NU - 1 and hp == 0:
                        # stage the hp0-heads O-projection partials inside
                        # the last attention block
                        filler.extend(oproj_a_item(u, qs) for qs in range(4))
            # tail: drain deferred AVs, keep the PE warm through the final
            # normalization chain, then finish the split O-projection
            av_drain()
            tail = list(filler)
            filler.clear()
            for i, fn in enumerate(tail):
                fn()
                if i in (0, 2, 4):
                    keep_warm(8)
            for qs in range(4):
                oproj_b_item(NU - 1, qs)()

    nc.compile()
    return nc


_PROGRAM = None


def _get_program():
    global _PROGRAM
    if _PROGRAM is None:
        _PROGRAM = build_program()
    return _PROGRAM


def kernel(**inputs):
    query = np.asarray(inputs["query"], dtype=np.float32)
    key = np.asarray(inputs["key"], dtype=np.float32)
    value = np.asarray(inputs["value"], dtype=np.float32)
    mask = np.asarray(inputs["mask"])
    Wq = np.asarray(inputs["Wq"], dtype=np.float32)
    Wk = np.asarray(inputs["Wk"], dtype=np.float32)
    Wv = np.asarray(inputs["Wv"], dtype=np.float32)
    Wo = np.asarray(inputs["Wo"], dtype=np.float32)
    bo = np.asarray(inputs["bo"], dtype=np.float32)
    rel_bias = np.asarray(inputs["rel_bias"], dtype=np.float32)

    if not np.all(mask != 0):
        raise NotImplementedError("kernel assumes an all-ones attention mask")

    nc = _get_program()
    scale = np.float32(1.0 / np.sqrt(DK))

    # sliding-window exp-bias table: ebt[p, lh, j] = exp(tbl[clip(895-j+p)])
    pp = np.arange(128)[:, None]
    jj = np.arange(1408)[None, :]
    widx = np.clip(895 - jj + pp, 0, 510)  # [128, 1408]

    in_maps = []
    for c in range(NCORES):
        b = c // 2
        hbase = (c % 2) * 4
        rows = slice(hbase * 64, (hbase + 4) * 64)

        wq_arr = np.ascontiguousarray(
            (Wq[rows, :] * scale).T.reshape(4, 128, 256).swapaxes(0, 1))
        wk_arr = np.ascontiguousarray(
            Wk[rows, :].T.reshape(4, 128, 256).swapaxes(0, 1))
        wv_arr = np.ascontiguousarray(
            Wv[rows, :].T.reshape(4, 128, 256).swapaxes(0, 1))

        wo_arr = np.empty((64, 4, 512), dtype=np.float32)
        ebt_arr = np.empty((128, 4, 1408), dtype=np.float16)
        cb_arr = np.zeros((128, 4, 3), dtype=np.float32)
        for lh in range(4):
            g = hbase + lh
            wo_arr[:, lh, :] = Wo[:, g * 64:(g + 1) * 64].T * (1.0 / 256.0)
            tbl = rel_bias[g]
            ebt_arr[:, lh, :] = np.exp(tbl)[widx]
            cb_arr[:, lh, 1] = tbl[0]
            cb_arr[:, lh, 2] = tbl[510]

        bf = np.float16
        in_maps.append({
            "xqT": np.ascontiguousarray(query[b].T).astype(bf),
            "xkT": np.ascontiguousarray(key[b].T).astype(bf),
            "xvT": np.ascontiguousarray(value[b].T).astype(bf),
            "wq": wq_arr.astype(bf), "wk": wk_arr.astype(bf),
            "wv": wv_arr.astype(bf), "wo": wo_arr.astype(bf),
            "ebt": ebt_arr, "cb": cb_arr,
        })

    res = run_bass_kernel_spmd(nc, in_maps, list(range(NCORES)), trace=False)

    out = np.zeros((B, S, D), dtype=np.float32)
    for c in range(NCORES):
        out[c // 2] += res.results[c]["out"]
    out += bo[None, None, :]
    return out


# revision 46
# speedup vs baseline: 1.0567x; 1.0003x over previous
"""Trainium2 Bass kernel for nn_MultiHeadAttention_34162169872901.

MultiHeadAttention (B=4, S=2048, d_model=512, 8 heads, d_k=64) with a
relative-position bias table (511 entries, clamp +-255) and an all-ones mask.

Sharding (8 NeuronCores): core c handles batch b = c//2 and 4 of the 8 heads
(c%2 selects the head half) -- data parallel on B, tensor parallel on heads.
Each core computes its 4 heads' Q/K/V projections, the full attention for its
batch, normalization, and its partial output projection; the host sums the two
partial outputs per batch (and adds the output bias bo).

v4 -- scheduled for the ACT (exp) bottleneck:
  - The softmax exp on the scalar/ACT engine is the binding resource
    (128 x [128,1024] exps ~= 1.13us each).  The schedule starts exp as
    early as possible and tries to never let ACT starve.
  - DMA triggers cost ~650ns each and serialize per engine, so inputs are
    consolidated into a few multi-block tiles, triggered first thing in
    the program, in need order, on the two fast queues (sync + gpsimd).
  - Score matmuls are emitted ah-major so the quadrant whose PSUM bank
    frees first is refilled without waiting for the second exp.
  - AV matmuls are globally deferred by two groups (and carry across
    block boundaries) so filler never blocks the score stream and the
    next block's first scores precede the previous block's AV drain.
  - The relative-bias exp-table is a [128, 4, 1408] sliding-window table
    (every in-band Toeplitz block is a contiguous 512-col slice).
  - PSUM: scores ring 2x[128,1024] (4 banks), AV-accumulator ring
    2x[65,512] (2 banks), epilogue/projection ring 2x[128,512] (2 banks).
  - Per-(u,hp) normalization and the O-projection are pipelined into the
    following attention block; the final O-projection is split so half is
    staged during the last block, with keep-warm matmuls bridging the
    final normalization chain.
"""

import sys
import types

import numpy as np

B = 4
S = 2048
D = 512
NHEAD = 8
DK = 64
NCORES = 8
MAX_REL = 255
NKT = S // 128   # 16 k-tiles
EXPA = float(2 ** 23) / float(np.log(2))      # Schraudolph exp: scale
EXPB = float(127 * 2 ** 23 - 482579)          # bias w/ HW-fit magic const
NU = S // 512    # 4 q-units
NG = NKT // 2    # 8 score groups per (u, hp)


def _install_axon_hooks():
    """Provide antenv.axon_hooks (missing in this image) so bass_utils'
    trace path can be used; harmless when tracing is off."""
    try:
        import antenv
    except ImportError:
        return
    try:
        from antenv.axon_hooks import get_axon_ntff_profile_hook  # noqa: F401
        return
    except ImportError:
        pass
    hook = None
    try:
        from trn_agent_boot.trn_boot import _ntff_profile_via_ctypes
        hook = _ntff_profile_via_ctypes("/opt/axon/libaxon_pjrt.so")
    except Exception:
        hook = None
    m = types.ModuleType("antenv.axon_hooks")
    m.get_axon_ntff_profile_hook = lambda: hook
    m.set_axon_ntff_profile_hook = lambda h: None
    sys.modules["antenv.axon_hooks"] = m
    antenv.axon_hooks = m


_install_axon_hooks()

import concourse.bass as bass  # noqa: E402
import concourse.bacc as bacc  # noqa: E402
import concourse.mybir as mybir  # noqa: E402
from concourse import tile  # noqa: E402
from concourse.bass_utils import run_bass_kernel_spmd  # noqa: E402
from concourse.vector_clock import ScopedClock as _ScopedClock  # noqa: E402

f32 = mybir.dt.float32
bf16 = mybir.dt.bfloat16
f16 = mybir.dt.float16
AF = mybir.ActivationFunctionType


def _patched_drain_and_barrier(self, tick_clock, wait_clock):
    # walrus in this container rejects >2 sem waits on one instruction; emit
    # the tail-drain waits as standalone wait instructions instead.
    nc = self.nc
    dummy = mybir.InstNoOp(name="drain-wait-probe", engine=mybir.EngineType.SP)
    wait_clock.add_sem_waits(dummy, _ScopedClock({None: tick_clock.global_clock}))
    handles = {h.name: h for h in self.sems.allocated().values()}
    si = dummy.sync_info
    for w in (si.on_wait if si is not None else []):
        nc.sync.wait_ge(handles[w.ant_name], w.wait_value)
    nc.sync.drain()
    nc.all_engine_barrier()
    popped = nc._tile_sem_poison_stack.pop()
    assert popped is self._sem_poison
    nc.clear_and_free_semaphores(list(self.sems.allocated().values()))
    nc.all_engine_barrier()


tile.TileContext._drain_and_barrier = _patched_drain_and_barrier


def _delta(t, u):
    # key-tile offset minus query-chunk offset; bias entry index is
    # delta + (p - f) + 255 clipped to [0, 510]
    return 128 * t - 512 * u


def _cls(t, u):
    d = _delta(t, u)
    if d <= -384:
        return 1  # whole block clamps to table[0]
    if d >= 768:
        return 2  # whole block clamps to table[510]
    return 0      # in-band: needs the Toeplitz block


def _ebt_col(t, u):
    # start column of the [128,512] in-band block inside the 1408-wide
    # sliding-window exp-bias table: 640 - delta
    return 640 - _delta(t, u)


def _gorder(u):
    # out-of-band groups first (no bias-table dependency, constant folded
    # into the exp bias).  For u == 0, order by K-projection availability:
    # high k-columns first, in-band groups in descending g.
    if u == 0:
        return [4, 5, 6, 7, 3, 2, 1, 0]
    def key(g):
        return (_cls(2 * g, u) == 0, g)
    return sorted(range(NG), key=key)


def build_program():
    nc = bacc.Bacc()

    xqT = nc.declare_dram_parameter("xqT", [D, S], f16, isOutput=False)
    xkT = nc.declare_dram_parameter("xkT", [D, S], f16, isOutput=False)
    xvT = nc.declare_dram_parameter("xvT", [D, S], f16, isOutput=False)
    wq = nc.declare_dram_parameter("wq", [128, 4, 256], f16, isOutput=False)
    wk = nc.declare_dram_parameter("wk", [128, 4, 256], f16, isOutput=False)
    wv = nc.declare_dram_parameter("wv", [128, 4, 256], f16, isOutput=False)
    wo = nc.declare_dram_parameter("wo", [64, 4, 512], f16, isOutput=False)
    ebtd = nc.declare_dram_parameter("ebt", [128, 4, 1408], f16, isOutput=False)
    cbd = nc.declare_dram_parameter("cb", [128, 4, 3], f32, isOutput=False)
    cbid = nc.declare_dram_parameter("cbi", [128, 4, 3], f32, isOutput=False)
    outd = nc.declare_dram_parameter("out", [S, D], f32, isOutput=True)

    with tile.TileContext(nc) as tc:
        with (
            tc.tile_pool(name="sb", bufs=1) as pool,
            tc.tile_pool(name="xt", bufs=1) as xpool,
            tc.tile_pool(name="pt", bufs=14) as ppool,
            tc.tile_pool(name="cxp", bufs=2) as cpool,
            tc.tile_pool(name="obp", bufs=4) as opool,
            tc.tile_pool(name="sc", bufs=2, space="PSUM") as scp,
            tc.tile_pool(name="cx", bufs=2, space="PSUM") as ctxpool,
            tc.tile_pool(name="ep", bufs=2, space="PSUM") as epp,
        ):
            # ---- persistent SBUF tiles -------------------------------------
            wq_sb = pool.tile([128, 4, 256], f16, tag="wq")
            wk_sb = pool.tile([128, 4, 256], f16, tag="wk")
            wv_sb = pool.tile([128, 4, 256], f16, tag="wv")
            wo_sb = pool.tile([64, 4, 512], f16, tag="wo")
            ebt_sb = pool.tile([128, 4, 1408], f16, tag="ebt")
            cb_sb = pool.tile([128, 4, 3], f32, tag="cb")
            cbi_sb = pool.tile([128, 4, 3], f32, tag="cbi")
            qt_sb = pool.tile([128, 2, S], f16, tag="qt")
            kt_sb = pool.tile([128, 2, S], f16, tag="kt")
            v_sb = pool.tile([128, NKT, 4 * 65], f16, tag="v")
            ones_c = pool.tile([1, 64], f16, tag="ones")
            warm = pool.tile([128, 16], f32, tag="warm")

            # input tiles, consolidated so each is one DMA trigger:
            # xq0: q-unit 0 for all 4 d-blocks; xq123: q-units 1..3;
            # xka: k cols 1024:2048; xkbc: k cols 0:1024; xv01/xv23: V halves
            xq0 = xpool.tile([128, 4, 512], f16, tag="xq0")
            xq123 = xpool.tile([128, 4, 1536], f16, tag="xq123")
            xka = xpool.tile([128, 4, 1024], f16, tag="xka")
            xkbc = xpool.tile([128, 4, 1024], f16, tag="xkbc")
            xv01 = xpool.tile([128, 2, 2048], f16, tag="xv01")
            xv23 = xpool.tile([128, 2, 2048], f16, tag="xv23")

            def blk(t, cols):
                return t[:, cols].rearrange("(c p) s -> p c s", p=128)

            # ---- DMA triggers first (each costs ~650ns of engine time and
            # the data cannot start moving until its trigger runs) ----------
            nc.sync.dma_start(wq_sb[:], wq[:])
            nc.sync.dma_start(wv_sb[:], wv[:])
            nc.sync.dma_start(xq0[:], blk(xqT, slice(0, 512)))
            nc.sync.dma_start(xv01[:],
                              xvT[0:256, :].rearrange("(c p) s -> p c s", p=128))
            nc.sync.dma_start(xkbc[:], blk(xkT, slice(0, 1024)))
            nc.sync.dma_start(xq123[:], blk(xqT, slice(512, 2048)))
            nc.sync.dma_start(wo_sb[:], wo[:])
            nc.gpsimd.dma_start(wk_sb[:], wk[:])
            nc.gpsimd.dma_start(cb_sb[:], cbd[:])
            nc.gpsimd.dma_start(cbi_sb[:], cbid[:])
            nc.gpsimd.dma_start(xka[:], blk(xkT, slice(1024, 2048)))
            nc.gpsimd.dma_start(xv23[:],
                                xvT[256:512, :].rearrange("(c p) s -> p c s",
                                                          p=128))
            nc.gpsimd.dma_start(ebt_sb[:], ebtd[:])

            # load the exp table set (one-time ~2.7us) while DMAs stream
            nc.vector.memset(warm[:], 0.0)
            nc.scalar.activation(warm[:], warm[:], AF.Exp, bias=0.0, scale=1.0)
            nc.vector.memset(ones_c[:], 1.0)

            def xv_slice(ct, cols):
                t = (xv01, xv23)[ct // 2]
                return t[:, ct % 2, cols]

            # ---- projection helpers ----------------------------------------
            def k_rhs(sc, ct):
                if sc < 2:
                    return xkbc[:, ct, sc * 512:(sc + 1) * 512]
                return xka[:, ct, (sc - 2) * 512:(sc - 1) * 512]

            def q_rhs(sc, ct):
                if sc == 0:
                    return xq0[:, ct, :]
                return xq123[:, ct, (sc - 1) * 512:sc * 512]

            def proj_group(w_sb, rhs_fn, dst, hp, sc, ptag):
                pk = (ctxpool if ptag == "cx" else epp).tile(
                    [128, 512], f32, tag=ptag, name=f"pj{hp}{sc}")
                for ct in range(4):
                    nc.tensor.matmul(
                        pk[:],
                        lhsT=w_sb[:, ct, hp * 128:(hp + 1) * 128],
                        rhs=rhs_fn(sc, ct),
                        start=(ct == 0), stop=(ct == 3),
                    )
                nc.vector.tensor_copy(dst[:, hp, sc * 512:(sc + 1) * 512], pk[:])

            # pre-loop projections: Q hp0 unit-0 and K hp0 high columns
            proj_group(wq_sb, q_rhs, qt_sb, 0, 0, "ep")
            proj_group(wk_sb, k_rhs, kt_sb, 0, 2, "cx")
            proj_group(wk_sb, k_rhs, kt_sb, 0, 3, "cx")

            def v_group(g):
                def emit():
                    pv = epp.tile([128, 512], f32, tag="ep", name=f"pv{g}")
                    for sti in range(2):
                        st = 2 * g + sti
                        for ct in range(4):
                            nc.tensor.matmul(
                                pv[:, sti * 256:sti * 256 + 256],
                                lhsT=xv_slice(ct, slice(st * 128, (st + 1) * 128)),
                                rhs=wv_sb[:, ct, :],
                                start=(ct == 0), stop=(ct == 3),
                            )
                    for sti in range(2):
                        st = 2 * g + sti
                        vslice = v_sb[:, st, :].rearrange(
                            "p (h x) -> p h x", x=65)
                        nc.vector.tensor_copy(
                            vslice[:, :, 0:64],
                            pv[:, sti * 256:sti * 256 + 256].rearrange(
                                "p (h x) -> p h x", x=64),
                        )
                        nc.vector.memset(vslice[:, :, 64:65], 1.0)
                return emit

            def pg_item(w_sb, rhs_fn, dst, hp, sc):
                def emit():
                    proj_group(w_sb, rhs_fn, dst, hp, sc, "ep")
                return emit

            def noop():
                pass

            # ---- attention + pipelined epilogue ----------------------------
            cx_tiles = {}     # (u, lh) -> normalized ctx [64, 512] f16
            ob_a = {}         # qs -> staged hp0 O-projection partial (f32)
            filler = []       # FIFO of emission closures
            avq = []          # globally deferred AV matmul emissions
            AV_LAG = 2

            def av_drain(n=None):
                k = len(avq) if n is None else n
                for _ in range(k):
                    if avq:
                        avq.pop(0)()

            def epilogue_items(u, hp, ctxps):
                """Normalization for the two heads of (u, hp), split into
                pipeline-friendly chunks (with no-op spacers so the serial
                chain never blocks an engine queue)."""
                state = {}

                def e1():
                    for ah in range(2):
                        ctxf = cpool.tile([65, 512], f32, tag="ctxf", bufs=4,
                                          name=f"ctxf{u}{hp}{ah}")
                        nc.vector.tensor_copy(ctxf[:], ctxps[ah][:])
                        state[ah] = ctxf

                def e2a():
                    lp = cpool.tile([1, 1024], f32, tag="lp", name=f"lp{u}{hp}")
                    nc.gpsimd.dma_start(lp[:, 0:512], state[0][64:65, :])
                    nc.gpsimd.dma_start(lp[:, 512:1024], state[1][64:65, :])
                    state["lp"] = lp

                def e2b():
                    linv = cpool.tile([1, 1024], f32, tag="linv",
                                      name=f"linv{u}{hp}")
                    nc.vector.reciprocal_approx_fast(linv[:], state["lp"][:])
                    linvb = cpool.tile([1, 1024], f16, tag="linvb",
                                       name=f"linvb{u}{hp}")
                    nc.vector.tensor_scalar_mul(linvb[:], linv[:], 256.0)
                    state["linvb"] = linvb

                def e3():
                    linvb = state["linvb"]
                    for ah in range(2):
                        bc = epp.tile([64, 512], f32, tag="ep",
                                      name=f"bc{u}{hp}{ah}")
                        nc.tensor.matmul(
                            bc[:], lhsT=ones_c[:],
                            rhs=linvb[:, ah * 512:(ah + 1) * 512],
                            start=True, stop=True)
                        cxn = cpool.tile([64, 512], f16, tag="cxn", bufs=8,
                                         name=f"cx{u}{hp}{ah}")
                        nc.vector.tensor_mul(cxn[:], bc[:], state[ah][0:64, :])
                        cx_tiles[(u, 2 * hp + ah)] = cxn

                return [e1, e2a, noop, e2b, noop, e3]

            def oproj_items(u):
                items = []
                for qs in range(4):
                    def emit(u=u, qs=qs):
                        po = epp.tile([128, 512], f32, tag="ep",
                                      name=f"po{u}{qs}")
                        for lh in range(4):
                            nc.tensor.matmul(
                                po[:],
                                lhsT=cx_tiles[(u, lh)][:, qs * 128:(qs + 1) * 128],
                                rhs=wo_sb[:, lh, :],
                                start=(lh == 0), stop=(lh == 3),
                            )
                        ob = opool.tile([128, 512], f32, tag="ob",
                                        name=f"ob{u}{qs}")
                        nc.vector.tensor_copy(ob[:], po[:])
                        nc.sync.dma_start(
                            outd[u * 512 + qs * 128: u * 512 + (qs + 1) * 128, :],
                            ob[:],
                        )
                    items.append(emit)
                return items

            # split O-projection for the last u: stage the hp0-heads partial
            # during the last attention block, finish + combine in the tail
            def oproj_a_item(u, qs):
                def emit():
                    po = epp.tile([128, 512], f32, tag="ep",
                                  name=f"poa{u}{qs}")
                    for lh in range(2):
                        nc.tensor.matmul(
                            po[:],
                            lhsT=cx_tiles[(u, lh)][:, qs * 128:(qs + 1) * 128],
                            rhs=wo_sb[:, lh, :],
                            start=(lh == 0), stop=(lh == 1),
                        )
                    oba = opool.tile([128, 512], f32, tag="oba", bufs=4,
                                     name=f"oba{u}{qs}")
                    nc.vector.tensor_copy(oba[:], po[:])
                    ob_a[qs] = oba
                return emit

            def oproj_b_item(u, qs):
                def emit():
                    po = epp.tile([128, 512], f32, tag="ep",
                                  name=f"pob{u}{qs}")
                    for lh in range(2, 4):
                        nc.tensor.matmul(
                            po[:],
                            lhsT=cx_tiles[(u, lh)][:, qs * 128:(qs + 1) * 128],
                            rhs=wo_sb[:, lh, :],
                            start=(lh == 2), stop=(lh == 3),
                        )
                    ob = opool.tile([128, 512], f32, tag="ob",
                                    name=f"ob{u}{qs}")
                    nc.vector.tensor_add(ob[:], po[:], ob_a[qs][:])
                    nc.sync.dma_start(
                        outd[u * 512 + qs * 128: u * 512 + (qs + 1) * 128, :],
                        ob[:],
                    )
                return emit

            def keep_warm(n=8):
                # junk matmuls bridging the tail normalization chain so HAM
                # doesn't re-throttle the PE before the final O-projection
                dz = ctxpool.tile([65, 512], f32, tag="cx", name="dz")
                for r in range(n):
                    vsl = v_sb[:, r, :].rearrange(
                        "p (h x) -> p h x", x=65)[:, 0, :]
                    nc.tensor.matmul(dz[:], lhsT=vsl, rhs=qt_sb[:, 0, 0:512],
                                     start=True, stop=True)

            def attention(u, hp, pre_sched=None, post_sched=None, post_rate=1,
                          dve_exp=False):
                """pre_sched/post_sched: {gi: [closures]} emitted before the
                scores (pre) or between the exps and the AV matmuls (post) of
                group gi.  post_rate: queued filler items popped at each post
                point.  AV matmuls are appended to the global deferred queue
                and drained AV_LAG groups later (carrying across blocks)."""
                ctxps = [
                    ctxpool.tile([65, 512], f32, tag="cx",
                                 name=f"ctxp{u}{hp}{i}")
                    for i in range(2)
                ]
                nav = [0, 0]

                def av_item(g, srcs):
                    def emit():
                        for ah in range(2):
                            lh = 2 * hp + ah
                            for ti in range(2):
                                t = 2 * g + ti
                                vsl = v_sb[:, t, :].rearrange(
                                    "p (h x) -> p h x", x=65)[:, lh, :]
                                nav[ah] += 1
                                nc.tensor.matmul(
                                    ctxps[ah][:],
                                    lhsT=vsl,
                                    rhs=srcs[ah][:, ti * 512:(ti + 1) * 512],
                                    start=(nav[ah] == 1),
                                    stop=(nav[ah] == NKT),
                                )
                    return emit

                for gi, g in enumerate(_gorder(u)):
                    for fn in (pre_sched or {}).get(gi, []):
                        fn()
                    cls = _cls(2 * g, u)
                    sct = [scp.tile([128, 1024], f32, tag="sc",
                                    name=f"sct{u}{hp}{g}{i}")
                           for i in range(2)]
                    # ah-major: refill the freed PSUM bank's quadrant first
                    for ah in range(2):
                        for ti in range(2):
                            t = 2 * g + ti
                            nc.tensor.matmul(
                                sct[ah][:, ti * 512:(ti + 1) * 512],
                                lhsT=kt_sb[ah * 64:(ah + 1) * 64, hp,
                                           t * 128:(t + 1) * 128],
                                rhs=qt_sb[ah * 64:(ah + 1) * 64, hp,
                                          u * 512:(u + 1) * 512],
                                start=True, stop=True,
                                tile_position=(ah * 64, 0),
                            )
                    srcs = []
                    for ah in range(2):
                        lh = 2 * hp + ah
                        pt = ppool.tile([128, 1024], f16, tag="pt",
                                        name=f"pt{u}{hp}{g}{ah}")
                        if dve_exp and gi == 0 and ah == 1:
                            # offload this (out-of-band) exp to the DVE:
                            # y = bitcast_f32(i32(s*A + (B + A*bias)))
                            ti32 = ppool.tile([128, 1024], mybir.dt.int32,
                                              tag="ti", bufs=2,
                                              name=f"ti{u}{hp}{g}")
                            nc.vector.tensor_scalar(
                                ti32[:], sct[ah][:], EXPA,
                                cbi_sb[:, lh, cls:cls + 1],
                                op0=mybir.AluOpType.mult,
                                op1=mybir.AluOpType.add)
                            nc.vector.tensor_copy(pt[:], ti32[:].bitcast(f32))
                        else:
                            nc.scalar.activation(
                                pt[:], sct[ah][:], AF.Exp,
                                bias=cb_sb[:, lh, cls:cls + 1], scale=1.0,
                            )
                        srcs.append(pt)
                    if gi == 0:
                        # the previous block's carried AV matmuls must all be
                        # emitted before its epilogue (a filler item below)
                        # can be: dependency tracking only covers emitted
                        # writers
                        av_drain()
                    for fn in (post_sched or {}).get(gi, []):
                        fn()
                    for _ in range(post_rate):
                        if filler:
                            filler.pop(0)()
                    if cls == 0:
                        for ah in range(2):
                            lh = 2 * hp + ah
                            pt = srcs[ah]
                            src = ppool.tile([128, 1024], f16, tag="src",
                                             bufs=8,
                                             name=f"src{u}{hp}{g}{ah}")
                            for ti in range(2):
                                col = _ebt_col(2 * g + ti, u)
                                nc.vector.tensor_mul(
                                    src[:, ti * 512:(ti + 1) * 512],
                                    pt[:, ti * 512:(ti + 1) * 512],
                                    ebt_sb[:, lh, col:col + 512],
                                )
                            srcs[ah] = src
                    avq.append(av_item(g, srcs))
                    while len(avq) > AV_LAG:
                        avq.pop(0)()
                return ctxps

            # ---- block (0,0): explicit schedules ---------------------------
            # gorder(0) = [4,5,6,7,3,2,1,0]
            pre00 = {
                4: [pg_item(wk_sb, k_rhs, kt_sb, 0, 1)],
                6: [pg_item(wk_sb, k_rhs, kt_sb, 0, 0)],
            }
            post00 = {
                1: [v_group(4)],
                2: [v_group(5), v_group(6)],
                3: [v_group(7), v_group(3)],
                4: [v_group(2)],
                5: [v_group(1),
                    pg_item(wk_sb, k_rhs, kt_sb, 1, 0),
                    pg_item(wk_sb, k_rhs, kt_sb, 1, 1)],
                6: [v_group(0),
                    pg_item(wk_sb, k_rhs, kt_sb, 1, 2),
                    pg_item(wk_sb, k_rhs, kt_sb, 1, 3)],
            }
            ctxps = attention(0, 0, pre00, post00, post_rate=0)
            filler.extend([pg_item(wq_sb, q_rhs, qt_sb, 1, 1),
                           pg_item(wq_sb, q_rhs, qt_sb, 1, 2),
                           pg_item(wq_sb, q_rhs, qt_sb, 1, 3),
                           pg_item(wq_sb, q_rhs, qt_sb, 0, 1),
                           pg_item(wq_sb, q_rhs, qt_sb, 0, 2),
                           pg_item(wq_sb, q_rhs, qt_sb, 0, 3)])
            filler.extend(epilogue_items(0, 0, ctxps))

            # hp1 unit-0 Q projection must precede block (0,1)'s scores
            pre01 = {0: [pg_item(wq_sb, q_rhs, qt_sb, 1, 0)]}
            ctxps = attention(0, 1, pre01, None, post_rate=2, dve_exp=True)
            filler.extend(epilogue_items(0, 1, ctxps))
            filler.extend(oproj_items(0))

            for u in range(1, NU):
                for hp in range(2):
                    last = (u == NU - 1 and hp == 1)
                    ctxps = attention(u, hp, post_rate=2 if last else 1,
                                      dve_exp=True)
                    filler.extend(epilogue_items(u, hp, ctxps))
                    if hp == 1 and not last:
                        filler.extend(oproj_items(u))
                    if u == NU - 1 and hp == 0:
                        # stage the hp0-heads O-projection partials inside
                        # the last attention block
                        filler.extend(oproj_a_item(u, qs) for qs in range(4))
            # tail: drain deferred AVs, keep the PE warm through the final
            # normalization chain, then finish the split O-projection
            av_drain()
            tail = list(filler)
            filler.clear()
            for i, fn in enumerate(tail):
                fn()
                if i in (0, 2, 4):
                    keep_warm(8)
            for qs in range(4):
                oproj_b_item(NU - 1, qs)()

    nc.compile()
    return nc


_PROGRAM = None


def _get_program():
    global _PROGRAM
    if _PROGRAM is None:
        _PROGRAM = build_program()
    return _PROGRAM


def kernel(**inputs):
    query = np.asarray(inputs["query"], dtype=np.float32)
    key = np.asarray(inputs["key"], dtype=np.float32)
    value = np.asarray(inputs["value"], dtype=np.float32)
    mask = np.asarray(inputs["mask"])
    Wq = np.asarray(inputs["Wq"], dtype=np.float32)
    Wk = np.asarray(inputs["Wk"], dtype=np.float32)
    Wv = np.asarray(inputs["Wv"], dtype=np.float32)
    Wo = np.asarray(inputs["Wo"], dtype=np.float32)
    bo = np.asarray(inputs["bo"], dtype=np.float32)
    rel_bias = np.asarray(inputs["rel_bias"], dtype=np.float32)

    if not np.all(mask != 0):
        raise NotImplementedError("kernel assumes an all-ones attention mask")

    nc = _get_program()
    scale = np.float32(1.0 / np.sqrt(DK))

    # sliding-window exp-bias table: ebt[p, lh, j] = exp(tbl[clip(895-j+p)])
    pp = np.arange(128)[:, None]
    jj = np.arange(1408)[None, :]
    widx = np.clip(895 - jj + pp, 0, 510)  # [128, 1408]

    in_maps = []
    for c in range(NCORES):
        b = c // 2
        hbase = (c % 2) * 4
        rows = slice(hbase * 64, (hbase + 4) * 64)

        wq_arr = np.ascontiguousarray(
            (Wq[rows, :] * scale).T.reshape(4, 128, 256).swapaxes(0, 1))
        wk_arr = np.ascontiguousarray(
            Wk[rows, :].T.reshape(4, 128, 256).swapaxes(0, 1))
        wv_arr = np.ascontiguousarray(
            Wv[rows, :].T.reshape(4, 128, 256).swapaxes(0, 1))

        wo_arr = np.empty((64, 4, 512), dtype=np.float32)
        cbi_arr = np.zeros((128, 4, 3), dtype=np.float32)
        ebt_arr = np.empty((128, 4, 1408), dtype=np.float16)
        cb_arr = np.zeros((128, 4, 3), dtype=np.float32)
        for lh in range(4):
            g = hbase + lh
            wo_arr[:, lh, :] = Wo[:, g * 64:(g + 1) * 64].T * (1.0 / 256.0)
            tbl = rel_bias[g]
            ebt_arr[:, lh, :] = np.exp(tbl)[widx]
            cb_arr[:, lh, 1] = tbl[0]
            cb_arr[:, lh, 2] = tbl[510]
            cbi_arr[:, lh, 0] = EXPB
            cbi_arr[:, lh, 1] = EXPB + EXPA * tbl[0]
            cbi_arr[:, lh, 2] = EXPB + EXPA * tbl[510]

        bf = np.float16
        in_maps.append({
            "xqT": np.ascontiguousarray(query[b].T).astype(bf),
            "xkT": np.ascontiguousarray(key[b].T).astype(bf),
            "xvT": np.ascontiguousarray(value[b].T).astype(bf),
            "wq": wq_arr.astype(bf), "wk": wk_arr.astype(bf),
            "wv": wv_arr.astype(bf), "wo": wo_arr.astype(bf),
            "ebt": ebt_arr, "cb": cb_arr, "cbi": cbi_arr,
        })

    res = run_bass_kernel_spmd(nc, in_maps, list(range(NCORES)), trace=False)

    out = np.zeros((B, S, D), dtype=np.float32)
    for c in range(NCORES):
        out[c // 2] += res.results[c]["out"]
    out += bo[None, None, :]
    return out
